# revision 22
# baseline (speedup 1.0000x reference)
"""Trainium2 Bass kernel for nn_DeepSetsEnsemble (segment_reduce).

Model: PhiNet (3x pointwise conv 16->128->128->64 with ReLU) over 524288
points, uniform segment-mean into 4096 events of 128 points, concat with
per-event features [4096, 32], then MLP 96->256->128->1 with training-mode
BatchNorm after the first two layers and a final sigmoid.

Distribution: data-parallel over events. Core c owns events
[512c, 512c+512) = points [65536c, 65536(c+1)). Params replicated.
BatchNorm needs full-batch statistics -> two tiny AllReduces (512/256
floats) across the 8 cores.

Device layout choices:
 - "p-major" point order per core (host-side relayout): tile t holds point
   index-within-event p=t for all 512 events. The L3 output tile is then
   [features, events], so the segment-sum becomes a PSUM-accumulated chain
   of contiguous matmuls (lhsT = stacked identities / 128), with no
   strided reads and no pipeline tail.
 - fp32r matmuls for layers 1/2 and the MLP (full PE rate, fp32 storage).
   Layer 3 and the segment-sum run in bf16: walrus rejects fp32r
   col-tiling, and M=64 needs col-tiling to pack two point-tiles into the
   128 PSUM partitions so drains use all lanes.
 - ReLU+bias drains PSUM->SBUF are the kernel bottleneck (1 elem/cycle/
   lane): split between ScalarE (activation) and VectorE (tensor_scalar
   add+max), biggest free dims PSUM banks allow.
 - bm1/bm2 are dropped: training-mode BN subtracts the batch mean, so any
   bias added before BN cancels exactly.
"""
import sys
sys.path.insert(0, "/opt/trn_rl_repo")
sys.path.insert(0, "/root/.axon_site/_ro/trn_rl_repo")

import numpy as np

# ---------------------------------------------------------------- constants
B = 4096
L = 524288
CIN = 16
DVEC = 32
PHI = [128, 128, 64]
MLP = [256, 128]
EPS = 1e-5

N_CORES = 8
E = B // N_CORES          # 512 events per core
P = L // B                # 128 points per event
LC = L // N_CORES         # 65536 points per core
N_TILES = LC // 512       # 128 point-tiles (one per p when p-major)
N_CHUNKS = 16             # 8 tiles per chunk
INV_B = 1.0 / B

_PROG = None  # compiled program cache (per process)


def _build_program():
    import concourse.bass as bass
    import concourse.tile as tile
    from concourse import bacc, mybir
    from concourse.alu_op_type import AluOpType
    from contextlib import ExitStack

    F32 = mybir.dt.float32
    F32R = mybir.dt.float32r
    BF16 = mybir.dt.bfloat16
    AF = mybir.ActivationFunctionType
    X = mybir.AxisListType.X

    nc = bacc.Bacc("TRN2", target_bir_lowering=False, debug=False,
                   num_devices=N_CORES)

    x_d = nc.dram_tensor("x", [64, 16384], BF16, kind="ExternalInput")
    xbt_d = nc.dram_tensor("xbt", [DVEC, E], F32R, kind="ExternalInput")
    w1j_d = nc.dram_tensor("w1j", [128, 128], BF16, kind="ExternalInput")
    w2t_d = nc.dram_tensor("w2t", [128, 128], BF16, kind="ExternalInput")
    w3t_d = nc.dram_tensor("w3t", [128, 64], BF16, kind="ExternalInput")
    segw_d = nc.dram_tensor("segw", [128, 64], BF16, kind="ExternalInput")
    wm1_d = nc.dram_tensor("wm1", [96, 256], F32R, kind="ExternalInput")
    wm2a_d = nc.dram_tensor("wm2a", [128, 128], F32R, kind="ExternalInput")
    wm2b_d = nc.dram_tensor("wm2b", [128, 128], F32R, kind="ExternalInput")
    wm3_d = nc.dram_tensor("wm3", [128, 1], F32R, kind="ExternalInput")
    b1_d = nc.dram_tensor("b1", [128, 1], F32, kind="ExternalInput")
    b2_d = nc.dram_tensor("b2", [128, 1], F32, kind="ExternalInput")
    b3p_d = nc.dram_tensor("b3p", [128, 1], F32, kind="ExternalInput")
    bng_d = nc.dram_tensor("bng", [128, 3], F32, kind="ExternalInput")
    bnb_d = nc.dram_tensor("bnb", [128, 3], F32, kind="ExternalInput")
    bm3_d = nc.dram_tensor("bm3", [1, 1], F32, kind="ExternalInput")
    out_d = nc.dram_tensor("out", [1, E], F32, kind="ExternalOutput")

    with tile.TileContext(nc) as tc, ExitStack() as ctx:
        wpool = ctx.enter_context(tc.tile_pool(name="wpool", bufs=1))
        xpool = ctx.enter_context(tc.tile_pool(name="xpool", bufs=3))
        hpool = ctx.enter_context(tc.tile_pool(name="hpool", bufs=3))
        h3pool = ctx.enter_context(tc.tile_pool(name="h3pool", bufs=6))
        mpool = ctx.enter_context(tc.tile_pool(name="mpool", bufs=1))
        pspool = ctx.enter_context(tc.tile_pool(name="pspool", bufs=1, space="PSUM"))
        dram = ctx.enter_context(tc.tile_pool(name="dram", bufs=1, space="DRAM"))

        # ---- weights / params to SBUF
        w1sb = wpool.tile([128, 128], BF16)
        w2sb = wpool.tile([128, 128], BF16)
        w3sb = wpool.tile([128, 64], BF16)
        segsb = wpool.tile([128, 64], BF16)
        wm1sb = wpool.tile([96, 256], F32R)
        wm2asb = wpool.tile([128, 128], F32R)
        wm2bsb = wpool.tile([128, 128], F32R)
        wm3sb = wpool.tile([128, 1], F32R)
        b1sb = wpool.tile([128, 1], F32)
        b2sb = wpool.tile([128, 1], F32)
        b3psb = wpool.tile([128, 1], F32)
        bngsb = wpool.tile([128, 3], F32)
        bnbsb = wpool.tile([128, 3], F32)
        bm3sb = wpool.tile([1, 1], F32)
        for sb, d in [(w1sb, w1j_d), (w2sb, w2t_d), (w3sb, w3t_d),
                      (segsb, segw_d), (wm1sb, wm1_d), (wm2asb, wm2a_d),
                      (wm2bsb, wm2b_d), (wm3sb, wm3_d), (b1sb, b1_d),
                      (b2sb, b2_d), (b3psb, b3p_d), (bngsb, bng_d),
                      (bnbsb, bnb_d), (bm3sb, bm3_d)]:
            nc.gpsimd.dma_start(sb[:], d[:])

        zt = mpool.tile([96, E], F32R)            # MLP input [means; xbt]
        nc.gpsimd.dma_start(zt[64:96, :], xbt_d[:])

        # force the sigmoid table set once, up front: it also contains
        # relu/square/copy, so no mid-kernel ACT_TABLE_LOAD ever fires.
        sgdum = wpool.tile([1, 1], F32)
        nc.scalar.activation(sgdum[:], bm3sb[:], AF.Sigmoid)

        # segment sums accumulate per-chunk in a rotating PSUM slot (dual
        # col-tiled chains), then a DVE add folds them into SBUF so all 8
        # banks stay available for the 4-slot drain rotation.
        segacc = mpool.tile([64, 1024], F32)

        # warm the collective path early (overlaps phi compute)
        warm_in = dram.tile([1, 8], F32)
        warm_out = dram.tile([1, 8], F32)
        wtile = wpool.tile([1, 8], F32)
        nc.gpsimd.memset(wtile[:], 0.0)
        nc.sync.dma_start(warm_in[:], wtile[:])
        nc.gpsimd.collective_compute(
            "AllReduce", AluOpType.add,
            replica_groups=[list(range(N_CORES))],
            ins=[warm_in.opt()], outs=[warm_out.opt()])

        def drain(engine, dst, src, bias):
            # relu(src + bias): one PSUM->SBUF pass
            if engine == "act":
                nc.scalar.activation(dst, src, AF.Relu, bias=bias, scale=1.0)
            else:
                nc.vector.tensor_scalar(dst, src, bias, 0.0,
                                        AluOpType.add, AluOpType.max)

        # rotating PSUM drain groups: [128, 1024] (2 banks) x 4 bufs = all 8
        # banks; every group is drained by exactly one engine, alternating,
        # so both ACT and DVE stay saturated while PE fills ahead.
        gcount = [0]

        def psum_group():
            g = pspool.tile([128, 1024], F32, name="grp", tag="grp", bufs=4)
            eng = "act" if gcount[0] % 2 == 0 else "dve"
            gcount[0] += 1
            return g, eng

        for k in range(N_CHUNKS):
            xt = xpool.tile([128, 1024], BF16)
            for j in range(4):
                nc.sync.dma_start(xt[32 * j:32 * j + 16, :],
                                  x_d[16 * j:16 * j + 16,
                                      1024 * k:1024 * (k + 1)])

            h1sb = hpool.tile([128, 4096], BF16, name="h1sb", tag="h1sb")
            h2sb = hpool.tile([128, 4096], BF16, name="h2sb", tag="h2sb")
            h3sb = hpool.tile([128, 2048], BF16, name="h3sb", tag="h3sb")

            # L1: tiles t = 4u+j; 4 row-tiled MMs per u -> 2 groups
            for u in range(2):
                ga, ea = psum_group()
                gb, eb = psum_group()
                for j in range(4):
                    dst = (ga if j < 2 else gb)
                    nc.tensor.matmul(
                        dst[:, 512 * (j % 2):512 * (j % 2 + 1)],
                        w1sb[32 * j:32 * j + 16, :],
                        xt[32 * j:32 * j + 16, 512 * u:512 * (u + 1)],
                        start=True, stop=True, tile_position=(32 * j, 0))
                t0 = 4 * u
                drain(ea, h1sb[:, 512 * t0:512 * (t0 + 2)], ga[:], b1sb[:])
                drain(eb, h1sb[:, 512 * (t0 + 2):512 * (t0 + 4)], gb[:], b1sb[:])

            # L2: pairs of tiles per group
            for half in range(4):
                g, e = psum_group()
                for i, t in enumerate((2 * half, 2 * half + 1)):
                    nc.tensor.matmul(
                        g[:, 512 * i:512 * (i + 1)],
                        w2sb[:], h1sb[:, 512 * t:512 * (t + 1)],
                        start=True, stop=True)
                drain(e, h2sb[:, 1024 * half:1024 * (half + 1)], g[:], b2sb[:])

            # L3: two col-tiled pairs per group
            for q in range(2):
                g, e = psum_group()
                for i in range(2):
                    pr = 2 * q + i
                    t0, t1 = 2 * pr, 2 * pr + 1
                    nc.tensor.matmul(g[0:64, 512 * i:512 * (i + 1)], w3sb[:],
                                     h2sb[:, 512 * t0:512 * (t0 + 1)],
                                     start=True, stop=True,
                                     tile_position=(0, 0))
                    nc.tensor.matmul(g[64:128, 512 * i:512 * (i + 1)], w3sb[:],
                                     h2sb[:, 512 * t1:512 * (t1 + 1)],
                                     start=True, stop=True,
                                     tile_position=(0, 64))
                drain(e, h3sb[:, 1024 * q:1024 * (q + 1)], g[:], b3psb[:])

            sg, _ = psum_group()
            for pr in range(4):
                half = pr % 2
                nc.tensor.matmul(sg[0:64, 512 * half:512 * (half + 1)],
                                 segsb[:], h3sb[:, 512 * pr:512 * (pr + 1)],
                                 start=(pr < 2), stop=(pr >= 2))
            if k == 0:
                nc.vector.tensor_copy(segacc[:], sg[0:64, :])
            else:
                nc.vector.tensor_add(segacc[:], segacc[:], sg[0:64, :])

        # ---- MLP tail (events stay sharded; BN stats via AllReduce)
        nc.vector.tensor_add(zt[0:64, :], segacc[:, 0:512], segacc[:, 512:1024])

        y1p = pspool.tile([128, 1024], F32, name="y1p", tag="grp", bufs=4)
        stats = mpool.tile([128, 4], F32)
        sq_scr = mpool.tile([128, 1024], F32)
        for h in range(2):
            nc.tensor.matmul(y1p[:, 512 * h:512 * (h + 1)],
                             wm1sb[:, 128 * h:128 * (h + 1)], zt[:],
                             start=True, stop=True)
        # cross the engines over the two halves so they run in parallel
        nc.vector.tensor_reduce(stats[:, 0:1], y1p[:, 0:512], X, AluOpType.add)
        nc.scalar.activation(sq_scr[:, 512:1024], y1p[:, 512:1024],
                             AF.Square, accum_out=stats[:, 3:4])
        nc.vector.tensor_reduce(stats[:, 1:2], y1p[:, 512:1024], X, AluOpType.add)
        nc.scalar.activation(sq_scr[:, 0:512], y1p[:, 0:512],
                             AF.Square, accum_out=stats[:, 2:3])

        ar1_in = dram.tile([128, 4], F32)
        ar1_out = dram.tile([128, 4], F32)
        nc.sync.dma_start(ar1_in[:], stats[:])
        nc.gpsimd.collective_compute(
            "AllReduce", AluOpType.add,
            replica_groups=[list(range(N_CORES))],
            ins=[ar1_in.opt()], outs=[ar1_out.opt()])
        gst = mpool.tile([128, 4], F32)
        nc.sync.dma_start(gst[:], ar1_out[:])

        # scale/shift [128, 3]: cols 0,1 -> BN1 halves, col 2 -> BN2
        I32 = mybir.dt.int32
        mu = mpool.tile([128, 3], F32)
        var = mpool.tile([128, 3], F32)
        sd = mpool.tile([128, 3], F32)
        rs = mpool.tile([128, 3], F32)
        sc = mpool.tile([128, 3], F32)
        sh = mpool.tile([128, 3], F32)
        magic = mpool.tile([128, 3], I32)
        nc.gpsimd.memset(magic[:], 0x5F3759DF)

        def bn_params(c0, c1, sum_ap, sq_ap):
            # mu = sum/B ; var = sq/B - mu^2 ; sc = g * rsqrt(var+eps)
            # sh = be - mu*sc    (written into cols [c0:c1) of the tiles)
            # rsqrt: DVE-only (quake seed + 3 Newton steps) so no ACT sqrt
            # table set is ever needed.
            nc.vector.tensor_scalar_mul(mu[:, c0:c1], sum_ap, INV_B)
            nc.vector.tensor_scalar_mul(var[:, c0:c1], sq_ap, INV_B)
            nc.vector.tensor_mul(sd[:, c0:c1], mu[:, c0:c1], mu[:, c0:c1])
            nc.vector.tensor_sub(var[:, c0:c1], var[:, c0:c1], sd[:, c0:c1])
            u = var[:, c0:c1]
            nc.vector.tensor_scalar_add(u, u, EPS)
            r = rs[:, c0:c1]
            nc.vector.tensor_scalar(r.bitcast(I32), u.bitcast(I32), 1, None,
                                    AluOpType.arith_shift_right)
            nc.vector.tensor_sub(r.bitcast(I32), magic[:, c0:c1],
                                 r.bitcast(I32))
            t = sd[:, c0:c1]
            for _ in range(2):
                nc.vector.tensor_mul(t, r, r)
                nc.vector.tensor_mul(t, t, u)
                nc.vector.tensor_scalar(t, t, -0.5, 1.5,
                                        AluOpType.mult, AluOpType.add)
                nc.vector.tensor_mul(r, r, t)
            nc.vector.tensor_mul(sc[:, c0:c1], bngsb[:, c0:c1], r)
            nc.vector.tensor_mul(sh[:, c0:c1], mu[:, c0:c1], sc[:, c0:c1])
            nc.vector.tensor_sub(sh[:, c0:c1], bnbsb[:, c0:c1], sh[:, c0:c1])

        bn_params(0, 2, gst[:, 0:2], gst[:, 2:4])

        z1 = mpool.tile([128, 1024], F32R)
        for h in range(2):
            nc.scalar.activation(z1[:, 512 * h:512 * (h + 1)],
                                 y1p[:, 512 * h:512 * (h + 1)], AF.Relu,
                                 bias=sh[:, h:h + 1], scale=sc[:, h:h + 1])

        y2p = pspool.tile([128, 512], F32, name="y2p", tag="grp", bufs=4)
        nc.tensor.matmul(y2p[:], wm2asb[:], z1[:, 0:512], start=True, stop=False)
        nc.tensor.matmul(y2p[:], wm2bsb[:], z1[:, 512:1024], start=False, stop=True)
        st2 = mpool.tile([128, 2], F32)
        nc.vector.tensor_reduce(st2[:, 0:1], y2p[:], X, AluOpType.add)
        nc.scalar.activation(sq_scr[:, 0:512], y2p[:], AF.Square,
                             accum_out=st2[:, 1:2])

        ar2_in = dram.tile([128, 2], F32)
        ar2_out = dram.tile([128, 2], F32)
        nc.sync.dma_start(ar2_in[:], st2[:])
        nc.gpsimd.collective_compute(
            "AllReduce", AluOpType.add,
            replica_groups=[list(range(N_CORES))],
            ins=[ar2_in.opt()], outs=[ar2_out.opt()])
        gst2 = mpool.tile([128, 2], F32)
        nc.sync.dma_start(gst2[:], ar2_out[:])

        bn_params(2, 3, gst2[:, 0:1], gst2[:, 1:2])

        z2 = mpool.tile([128, 512], F32R)
        nc.scalar.activation(z2[:], y2p[:], AF.Relu,
                             bias=sh[:, 2:3], scale=sc[:, 2:3])

        lp = pspool.tile([1, 512], F32, name="lp", tag="grp", bufs=4)
        nc.tensor.matmul(lp[:], wm3sb[:], z2[:], start=True, stop=True)
        probs = mpool.tile([1, 512], F32)
        nc.scalar.activation(probs[:], lp[:], AF.Sigmoid,
                             bias=bm3sb[:], scale=1.0)
        nc.sync.dma_start(out_d[:], probs[:])

    nc.compile()
    return nc


# ------------------------------------------------------------- host helpers
def _prep_core_x(x_core):
    # x_core [16, 65536] fp32, natural order col = e*128 + p.
    # Output [64, 16384]: row 16j+c, col 1024k+512u+e  <->  (c, e, p=8k+4u+j)
    A = np.ascontiguousarray(x_core).reshape(CIN, E, P)
    A = A.reshape(CIN, E, 16, 2, 4)            # c, e, k, u, j
    Bm = A.transpose(4, 0, 2, 3, 1)            # j, c, k, u, e
    import ml_dtypes
    return np.ascontiguousarray(Bm.reshape(64, 16384)).astype(ml_dtypes.bfloat16)


def _numpy_reference(x_a, x_b, sample_indices, W1, b1, W2, b2, W3, b3,
                     Wm1, bm1, g1, be1, Wm2, bm2, g2, be2, Wm3, bm3):
    counts = sample_indices[0].astype(np.int64)
    bounds = np.concatenate([[0], np.cumsum(counts)])
    h = x_a[0]
    h = np.maximum(W1 @ h + b1[:, None], 0.0)
    h = np.maximum(W2 @ h + b2[:, None], 0.0)
    h = np.maximum(W3 @ h + b3[:, None], 0.0)
    Lh = h.shape[1]
    seg = np.searchsorted(bounds, np.arange(Lh), side="right") - 1
    sums = np.zeros((counts.shape[0], h.shape[0]), np.float32)
    valid = (seg >= 0) & (seg < counts.shape[0])
    np.add.at(sums, seg[valid], h.T[valid])
    means = sums / np.maximum(counts, 1)[:, None]
    z = np.concatenate([means, x_b.reshape(-1, DVEC)], axis=1)

    def bn(x, g, b):
        m = x.mean(0)
        v = x.var(0)
        return (x - m) / np.sqrt(v + EPS) * g + b

    z = np.maximum(bn(z @ Wm1.T + bm1, g1, be1), 0.0)
    z = np.maximum(bn(z @ Wm2.T + bm2, g2, be2), 0.0)
    logit = z @ Wm3.T + bm3
    return (1.0 / (1.0 + np.exp(-logit))).astype(np.float32)


LAST = {"exec_ns": None, "profile": None, "trace": None}


def kernel(_trace=False, **inputs):
    import ml_dtypes
    global _PROG

    x_a = np.asarray(inputs["x_a"], np.float32)
    x_b = np.asarray(inputs["x_b"], np.float32)
    si = np.asarray(inputs["sample_indices"])
    W1 = np.asarray(inputs["W1"], np.float32); b1 = np.asarray(inputs["b1"], np.float32)
    W2 = np.asarray(inputs["W2"], np.float32); b2 = np.asarray(inputs["b2"], np.float32)
    W3 = np.asarray(inputs["W3"], np.float32); b3 = np.asarray(inputs["b3"], np.float32)
    Wm1 = np.asarray(inputs["Wm1"], np.float32); bm1 = np.asarray(inputs["bm1"], np.float32)
    g1 = np.asarray(inputs["g1"], np.float32); be1 = np.asarray(inputs["be1"], np.float32)
    Wm2 = np.asarray(inputs["Wm2"], np.float32); bm2 = np.asarray(inputs["bm2"], np.float32)
    g2 = np.asarray(inputs["g2"], np.float32); be2 = np.asarray(inputs["be2"], np.float32)
    Wm3 = np.asarray(inputs["Wm3"], np.float32); bm3 = np.asarray(inputs["bm3"], np.float32)

    counts = si[0].astype(np.int64)
    if not (x_a.shape == (1, CIN, L) and x_b.shape == (B, DVEC)
            and np.all(counts == P)):
        # segmentation differs from the uniform layout this kernel is
        # specialized for; fall back to a host reference implementation.
        return _numpy_reference(x_a, x_b, si, W1, b1, W2, b2, W3, b3,
                                Wm1, bm1, g1, be1, Wm2, bm2, g2, be2,
                                Wm3, bm3)

    from concourse.bass_utils import run_bass_kernel_spmd

    if _PROG is None:
        _PROG = _build_program()

    w1j = np.zeros((128, 128), np.float32)
    for j in range(4):
        w1j[32 * j:32 * j + 16, :] = W1.T
    w1j = w1j.astype(ml_dtypes.bfloat16)
    eye = np.eye(64, dtype=np.float32)
    segw = (np.vstack([eye, eye]) / P).astype(ml_dtypes.bfloat16)
    shared = {
        "w1j": w1j,
        "w2t": np.ascontiguousarray(W2.T).astype(ml_dtypes.bfloat16),
        "w3t": np.ascontiguousarray(W3.T).astype(ml_dtypes.bfloat16),
        "segw": segw,
        "wm1": np.ascontiguousarray(Wm1.T),
        "wm2a": np.ascontiguousarray(Wm2.T[0:128]),
        "wm2b": np.ascontiguousarray(Wm2.T[128:256]),
        "wm3": np.ascontiguousarray(Wm3.T),
        "b1": b1.reshape(128, 1),
        "b2": b2.reshape(128, 1),
        "b3p": np.concatenate([b3, b3]).reshape(128, 1),
        "bng": np.stack([g1[0:128], g1[128:256], g2], axis=1).astype(np.float32),
        "bnb": np.stack([be1[0:128], be1[128:256], be2], axis=1).astype(np.float32),
        "bm3": bm3.reshape(1, 1),
    }
    in_maps = []
    for c in range(N_CORES):
        m = dict(shared)
        m["x"] = _prep_core_x(x_a[0, :, LC * c:LC * (c + 1)])
        m["xbt"] = np.ascontiguousarray(x_b[E * c:E * (c + 1)].T)
        in_maps.append(m)

    res = run_bass_kernel_spmd(_PROG, in_maps, list(range(N_CORES)),
                               trace=_trace)
    if _trace:
        LAST["exec_ns"] = res.exec_time_ns
        LAST["profile"] = res.profile_json
        LAST["trace"] = (res.instructions_and_trace[1]
                         if res.instructions_and_trace else None)
    out = np.concatenate([res.results[c]["out"][0] for c in range(N_CORES)])
    return out.reshape(B, 1).astype(np.float32)


# revision 23
# speedup vs baseline: 1.0412x; 1.0412x over previous
"""Trainium2 Bass kernel for nn_DeepSetsEnsemble (segment_reduce).

Model: PhiNet (3x pointwise conv 16->128->128->64 with ReLU) over 524288
points, uniform segment-mean into 4096 events of 128 points, concat with
per-event features [4096, 32], then MLP 96->256->128->1 with training-mode
BatchNorm after the first two layers and a final sigmoid.

Distribution: data-parallel over events. Core c owns events
[512c, 512c+512) = points [65536c, 65536(c+1)). Params replicated.
BatchNorm needs full-batch statistics -> two tiny AllReduces (512/256
floats) across the 8 cores.

Device layout choices:
 - "p-major" point order per core (host-side relayout): tile t holds point
   index-within-event p=t for all 512 events. The L3 output tile is then
   [features, events], so the segment-sum becomes a PSUM-accumulated chain
   of contiguous matmuls (lhsT = stacked identities / 128), with no
   strided reads and no pipeline tail.
 - fp32r matmuls for layers 1/2 and the MLP (full PE rate, fp32 storage).
   Layer 3 and the segment-sum run in bf16: walrus rejects fp32r
   col-tiling, and M=64 needs col-tiling to pack two point-tiles into the
   128 PSUM partitions so drains use all lanes.
 - ReLU+bias drains PSUM->SBUF are the kernel bottleneck (1 elem/cycle/
   lane): split between ScalarE (activation) and VectorE (tensor_scalar
   add+max), biggest free dims PSUM banks allow.
 - bm1/bm2 are dropped: training-mode BN subtracts the batch mean, so any
   bias added before BN cancels exactly.
"""
import sys
sys.path.insert(0, "/opt/trn_rl_repo")
sys.path.insert(0, "/root/.axon_site/_ro/trn_rl_repo")

import numpy as np

# ---------------------------------------------------------------- constants
B = 4096
L = 524288
CIN = 16
DVEC = 32
PHI = [128, 128, 64]
MLP = [256, 128]
EPS = 1e-5

N_CORES = 8
E = B // N_CORES          # 512 events per core
P = L // B                # 128 points per event
LC = L // N_CORES         # 65536 points per core
N_TILES = LC // 512       # 128 point-tiles (one per p when p-major)
N_CHUNKS = 16             # 8 tiles per chunk
INV_B = 1.0 / B

_PROG = None  # compiled program cache (per process)


def _build_program():
    import concourse.bass as bass
    import concourse.tile as tile
    from concourse import bacc, mybir
    from concourse.alu_op_type import AluOpType
    from contextlib import ExitStack

    F32 = mybir.dt.float32
    F32R = mybir.dt.float32r
    BF16 = mybir.dt.bfloat16
    AF = mybir.ActivationFunctionType
    X = mybir.AxisListType.X

    nc = bacc.Bacc("TRN2", target_bir_lowering=False, debug=False,
                   num_devices=N_CORES)

    x_d = nc.dram_tensor("x", [64, 16384], BF16, kind="ExternalInput")
    xbt_d = nc.dram_tensor("xbt", [DVEC, E], F32R, kind="ExternalInput")
    w1j_d = nc.dram_tensor("w1j", [128, 128], BF16, kind="ExternalInput")
    w2t_d = nc.dram_tensor("w2t", [128, 128], BF16, kind="ExternalInput")
    w3t_d = nc.dram_tensor("w3t", [128, 64], BF16, kind="ExternalInput")
    segw_d = nc.dram_tensor("segw", [128, 64], BF16, kind="ExternalInput")
    wm1_d = nc.dram_tensor("wm1", [96, 256], F32R, kind="ExternalInput")
    wm2a_d = nc.dram_tensor("wm2a", [128, 128], F32R, kind="ExternalInput")
    wm2b_d = nc.dram_tensor("wm2b", [128, 128], F32R, kind="ExternalInput")
    wm3_d = nc.dram_tensor("wm3", [128, 1], F32R, kind="ExternalInput")
    b1_d = nc.dram_tensor("b1", [128, 1], F32, kind="ExternalInput")
    b2_d = nc.dram_tensor("b2", [128, 1], F32, kind="ExternalInput")
    b3p_d = nc.dram_tensor("b3p", [128, 1], F32, kind="ExternalInput")
    bng_d = nc.dram_tensor("bng", [128, 3], F32, kind="ExternalInput")
    bnb_d = nc.dram_tensor("bnb", [128, 3], F32, kind="ExternalInput")
    bm3_d = nc.dram_tensor("bm3", [1, 1], F32, kind="ExternalInput")
    out_d = nc.dram_tensor("out", [1, E], F32, kind="ExternalOutput")

    with tile.TileContext(nc) as tc, ExitStack() as ctx:
        wpool = ctx.enter_context(tc.tile_pool(name="wpool", bufs=1))
        xpool = ctx.enter_context(tc.tile_pool(name="xpool", bufs=3))
        hpool = ctx.enter_context(tc.tile_pool(name="hpool", bufs=3))
        h3pool = ctx.enter_context(tc.tile_pool(name="h3pool", bufs=6))
        mpool = ctx.enter_context(tc.tile_pool(name="mpool", bufs=1))
        pspool = ctx.enter_context(tc.tile_pool(name="pspool", bufs=1, space="PSUM"))
        dram = ctx.enter_context(tc.tile_pool(name="dram", bufs=1, space="DRAM"))

        # ---- weights / params to SBUF
        w1sb = wpool.tile([128, 128], BF16)
        w2sb = wpool.tile([128, 128], BF16)
        w3sb = wpool.tile([128, 64], BF16)
        segsb = wpool.tile([128, 64], BF16)
        wm1sb = wpool.tile([96, 256], F32R)
        wm2asb = wpool.tile([128, 128], F32R)
        wm2bsb = wpool.tile([128, 128], F32R)
        wm3sb = wpool.tile([128, 1], F32R)
        b1sb = wpool.tile([128, 1], F32)
        b2sb = wpool.tile([128, 1], F32)
        b3psb = wpool.tile([128, 1], F32)
        bngsb = wpool.tile([128, 3], F32)
        bnbsb = wpool.tile([128, 3], F32)
        bm3sb = wpool.tile([1, 1], F32)
        for sb, d in [(w1sb, w1j_d), (w2sb, w2t_d), (w3sb, w3t_d),
                      (segsb, segw_d), (wm1sb, wm1_d), (wm2asb, wm2a_d),
                      (wm2bsb, wm2b_d), (wm3sb, wm3_d), (b1sb, b1_d),
                      (b2sb, b2_d), (b3psb, b3p_d), (bngsb, bng_d),
                      (bnbsb, bnb_d), (bm3sb, bm3_d)]:
            nc.gpsimd.dma_start(sb[:], d[:])

        zt = mpool.tile([96, E], F32R)            # MLP input [means; xbt]
        nc.gpsimd.dma_start(zt[64:96, :], xbt_d[:])

        # force the sigmoid table set once, up front: it also contains
        # relu/square/copy, so no mid-kernel ACT_TABLE_LOAD ever fires.
        sgdum = wpool.tile([1, 1], F32)
        nc.scalar.activation(sgdum[:], bm3sb[:], AF.Sigmoid)

        # segment sums accumulate per-chunk in a rotating PSUM slot (dual
        # col-tiled chains), then a DVE add folds them into SBUF so all 8
        # banks stay available for the 4-slot drain rotation.
        segacc = mpool.tile([128, E], F32)
        segtmp = mpool.tile([64, E], F32)

        # warm the collective path early (overlaps phi compute)
        warm_in = dram.tile([1, 8], F32)
        warm_out = dram.tile([1, 8], F32)
        wtile = wpool.tile([1, 8], F32)
        nc.gpsimd.memset(wtile[:], 0.0)
        nc.sync.dma_start(warm_in[:], wtile[:])
        nc.gpsimd.collective_compute(
            "AllReduce", AluOpType.add,
            replica_groups=[list(range(N_CORES))],
            ins=[warm_in.opt()], outs=[warm_out.opt()])

        def drain(engine, dst, src, bias):
            # relu(src + bias): one PSUM->SBUF pass
            if engine == "act":
                nc.scalar.activation(dst, src, AF.Relu, bias=bias, scale=1.0)
            else:
                nc.vector.tensor_scalar(dst, src, bias, 0.0,
                                        AluOpType.add, AluOpType.max)

        # rotating PSUM drain groups: [128, 1024] (2 banks) x 4 bufs = all 8
        # banks; each group is drained by one engine, assigned greedily so
        # projected ACT/DVE loads stay balanced (DVE also absorbs the
        # per-chunk segment adds).
        load = {"act": 0.0, "dve": 0.0}

        def psum_group():
            g = pspool.tile([128, 1024], F32, name="grp", tag="grp", bufs=4)
            eng = "act" if load["act"] + 1147 <= load["dve"] + 1192 else "dve"
            load[eng] += 1147 if eng == "act" else 1192
            return g, eng

        for k in range(N_CHUNKS):
            xt = xpool.tile([128, 1024], BF16)
            for j in range(4):
                nc.sync.dma_start(xt[32 * j:32 * j + 16, :],
                                  x_d[16 * j:16 * j + 16,
                                      1024 * k:1024 * (k + 1)])

            h1sb = hpool.tile([128, 4096], BF16, name="h1sb", tag="h1sb")
            h2sb = hpool.tile([128, 4096], BF16, name="h2sb", tag="h2sb")
            h3sb = hpool.tile([128, 2048], BF16, name="h3sb", tag="h3sb")

            # L1: tiles t = 4u+j; 4 row-tiled MMs per u -> 2 groups
            for u in range(2):
                ga, ea = psum_group()
                gb, eb = psum_group()
                for j in range(4):
                    dst = (ga if j < 2 else gb)
                    nc.tensor.matmul(
                        dst[:, 512 * (j % 2):512 * (j % 2 + 1)],
                        w1sb[32 * j:32 * j + 16, :],
                        xt[32 * j:32 * j + 16, 512 * u:512 * (u + 1)],
                        start=True, stop=True, tile_position=(32 * j, 0))
                t0 = 4 * u
                drain(ea, h1sb[:, 512 * t0:512 * (t0 + 2)], ga[:], b1sb[:])
                drain(eb, h1sb[:, 512 * (t0 + 2):512 * (t0 + 4)], gb[:], b1sb[:])

            # L2: pairs of tiles per group
            for half in range(4):
                g, e = psum_group()
                for i, t in enumerate((2 * half, 2 * half + 1)):
                    nc.tensor.matmul(
                        g[:, 512 * i:512 * (i + 1)],
                        w2sb[:], h1sb[:, 512 * t:512 * (t + 1)],
                        start=True, stop=True)
                drain(e, h2sb[:, 1024 * half:1024 * (half + 1)], g[:], b2sb[:])

            # L3: two col-tiled pairs per group
            for q in range(2):
                g, e = psum_group()
                for i in range(2):
                    pr = 2 * q + i
                    t0, t1 = 2 * pr, 2 * pr + 1
                    nc.tensor.matmul(g[0:64, 512 * i:512 * (i + 1)], w3sb[:],
                                     h2sb[:, 512 * t0:512 * (t0 + 1)],
                                     start=True, stop=True,
                                     tile_position=(0, 0))
                    nc.tensor.matmul(g[64:128, 512 * i:512 * (i + 1)], w3sb[:],
                                     h2sb[:, 512 * t1:512 * (t1 + 1)],
                                     start=True, stop=True,
                                     tile_position=(0, 64))
                drain(e, h3sb[:, 1024 * q:1024 * (q + 1)], g[:], b3psb[:])

            sg, _ = psum_group()
            for pr in range(4):
                half = pr % 2
                nc.tensor.matmul(sg[64 * half:64 * (half + 1), 0:512],
                                 segsb[:], h3sb[:, 512 * pr:512 * (pr + 1)],
                                 start=(pr < 2), stop=(pr >= 2),
                                 tile_position=(0, 64 * half))
            if k == 0:
                nc.vector.tensor_copy(segacc[:], sg[:, 0:512])
            else:
                nc.vector.tensor_add(segacc[:], segacc[:], sg[:, 0:512])
            load["dve"] += 658

        # ---- MLP tail (events stay sharded; BN stats via AllReduce)
        nc.sync.dma_start(segtmp[:], segacc[64:128, :])
        nc.vector.tensor_add(zt[0:64, :], segacc[0:64, :], segtmp[:])

        y1p = pspool.tile([128, 1024], F32, name="y1p", tag="grp", bufs=4)
        stats = mpool.tile([128, 4], F32)
        sq_scr = mpool.tile([128, 1024], F32)
        for h in range(2):
            nc.tensor.matmul(y1p[:, 512 * h:512 * (h + 1)],
                             wm1sb[:, 128 * h:128 * (h + 1)], zt[:],
                             start=True, stop=True)
        # cross the engines over the two halves so they run in parallel
        nc.vector.tensor_reduce(stats[:, 0:1], y1p[:, 0:512], X, AluOpType.add)
        nc.scalar.activation(sq_scr[:, 512:1024], y1p[:, 512:1024],
                             AF.Square, accum_out=stats[:, 3:4])
        nc.vector.tensor_reduce(stats[:, 1:2], y1p[:, 512:1024], X, AluOpType.add)
        nc.scalar.activation(sq_scr[:, 0:512], y1p[:, 0:512],
                             AF.Square, accum_out=stats[:, 2:3])

        ar1_in = dram.tile([128, 4], F32)
        ar1_out = dram.tile([128, 4], F32)
        nc.sync.dma_start(ar1_in[:], stats[:])
        nc.gpsimd.collective_compute(
            "AllReduce", AluOpType.add,
            replica_groups=[list(range(N_CORES))],
            ins=[ar1_in.opt()], outs=[ar1_out.opt()])
        gst = mpool.tile([128, 4], F32)
        nc.sync.dma_start(gst[:], ar1_out[:])

        # scale/shift [128, 3]: cols 0,1 -> BN1 halves, col 2 -> BN2
        I32 = mybir.dt.int32
        mu = mpool.tile([128, 3], F32)
        var = mpool.tile([128, 3], F32)
        sd = mpool.tile([128, 3], F32)
        rs = mpool.tile([128, 3], F32)
        sc = mpool.tile([128, 3], F32)
        sh = mpool.tile([128, 3], F32)
        magic = mpool.tile([128, 3], I32)
        nc.gpsimd.memset(magic[:], 0x5F3759DF)

        def bn_params(c0, c1, sum_ap, sq_ap):
            # mu = sum/B ; var = sq/B - mu^2 ; sc = g * rsqrt(var+eps)
            # sh = be - mu*sc    (written into cols [c0:c1) of the tiles)
            # rsqrt: DVE-only (quake seed + 3 Newton steps) so no ACT sqrt
            # table set is ever needed.
            nc.vector.tensor_scalar_mul(mu[:, c0:c1], sum_ap, INV_B)
            nc.vector.tensor_scalar_mul(var[:, c0:c1], sq_ap, INV_B)
            nc.vector.tensor_mul(sd[:, c0:c1], mu[:, c0:c1], mu[:, c0:c1])
            nc.vector.tensor_sub(var[:, c0:c1], var[:, c0:c1], sd[:, c0:c1])
            u = var[:, c0:c1]
            nc.vector.tensor_scalar_add(u, u, EPS)
            r = rs[:, c0:c1]
            nc.vector.tensor_scalar(r.bitcast(I32), u.bitcast(I32), 1, None,
                                    AluOpType.arith_shift_right)
            nc.vector.tensor_sub(r.bitcast(I32), magic[:, c0:c1],
                                 r.bitcast(I32))
            t = sd[:, c0:c1]
            for _ in range(2):
                nc.vector.tensor_mul(t, r, r)
                nc.vector.tensor_mul(t, t, u)
                nc.vector.tensor_scalar(t, t, -0.5, 1.5,
                                        AluOpType.mult, AluOpType.add)
                nc.vector.tensor_mul(r, r, t)
            nc.vector.tensor_mul(sc[:, c0:c1], bngsb[:, c0:c1], r)
            nc.vector.tensor_mul(sh[:, c0:c1], mu[:, c0:c1], sc[:, c0:c1])
            nc.vector.tensor_sub(sh[:, c0:c1], bnbsb[:, c0:c1], sh[:, c0:c1])

        bn_params(0, 2, gst[:, 0:2], gst[:, 2:4])

        z1 = mpool.tile([128, 1024], F32R)
        for h in range(2):
            nc.scalar.activation(z1[:, 512 * h:512 * (h + 1)],
                                 y1p[:, 512 * h:512 * (h + 1)], AF.Relu,
                                 bias=sh[:, h:h + 1], scale=sc[:, h:h + 1])

        y2p = pspool.tile([128, 512], F32, name="y2p", tag="grp", bufs=4)
        nc.tensor.matmul(y2p[:], wm2asb[:], z1[:, 0:512], start=True, stop=False)
        nc.tensor.matmul(y2p[:], wm2bsb[:], z1[:, 512:1024], start=False, stop=True)
        st2 = mpool.tile([128, 2], F32)
        nc.vector.tensor_reduce(st2[:, 0:1], y2p[:], X, AluOpType.add)
        nc.scalar.activation(sq_scr[:, 0:512], y2p[:], AF.Square,
                             accum_out=st2[:, 1:2])

        ar2_in = dram.tile([128, 2], F32)
        ar2_out = dram.tile([128, 2], F32)
        nc.sync.dma_start(ar2_in[:], st2[:])
        nc.gpsimd.collective_compute(
            "AllReduce", AluOpType.add,
            replica_groups=[list(range(N_CORES))],
            ins=[ar2_in.opt()], outs=[ar2_out.opt()])
        gst2 = mpool.tile([128, 2], F32)
        nc.sync.dma_start(gst2[:], ar2_out[:])

        bn_params(2, 3, gst2[:, 0:1], gst2[:, 1:2])

        z2 = mpool.tile([128, 512], F32R)
        nc.scalar.activation(z2[:], y2p[:], AF.Relu,
                             bias=sh[:, 2:3], scale=sc[:, 2:3])

        lp = pspool.tile([1, 512], F32, name="lp", tag="grp", bufs=4)
        nc.tensor.matmul(lp[:], wm3sb[:], z2[:], start=True, stop=True)
        probs = mpool.tile([1, 512], F32)
        nc.scalar.activation(probs[:], lp[:], AF.Sigmoid,
                             bias=bm3sb[:], scale=1.0)
        nc.sync.dma_start(out_d[:], probs[:])

    nc.compile()
    return nc


# ------------------------------------------------------------- host helpers
def _prep_core_x(x_core):
    # x_core [16, 65536] fp32, natural order col = e*128 + p.
    # Output [64, 16384]: row 16j+c, col 1024k+512u+e  <->  (c, e, p=8k+4u+j)
    A = np.ascontiguousarray(x_core).reshape(CIN, E, P)
    A = A.reshape(CIN, E, 16, 2, 4)            # c, e, k, u, j
    Bm = A.transpose(4, 0, 2, 3, 1)            # j, c, k, u, e
    import ml_dtypes
    return np.ascontiguousarray(Bm.reshape(64, 16384)).astype(ml_dtypes.bfloat16)


def _numpy_reference(x_a, x_b, sample_indices, W1, b1, W2, b2, W3, b3,
                     Wm1, bm1, g1, be1, Wm2, bm2, g2, be2, Wm3, bm3):
    counts = sample_indices[0].astype(np.int64)
    bounds = np.concatenate([[0], np.cumsum(counts)])
    h = x_a[0]
    h = np.maximum(W1 @ h + b1[:, None], 0.0)
    h = np.maximum(W2 @ h + b2[:, None], 0.0)
    h = np.maximum(W3 @ h + b3[:, None], 0.0)
    Lh = h.shape[1]
    seg = np.searchsorted(bounds, np.arange(Lh), side="right") - 1
    sums = np.zeros((counts.shape[0], h.shape[0]), np.float32)
    valid = (seg >= 0) & (seg < counts.shape[0])
    np.add.at(sums, seg[valid], h.T[valid])
    means = sums / np.maximum(counts, 1)[:, None]
    z = np.concatenate([means, x_b.reshape(-1, DVEC)], axis=1)

    def bn(x, g, b):
        m = x.mean(0)
        v = x.var(0)
        return (x - m) / np.sqrt(v + EPS) * g + b

    z = np.maximum(bn(z @ Wm1.T + bm1, g1, be1), 0.0)
    z = np.maximum(bn(z @ Wm2.T + bm2, g2, be2), 0.0)
    logit = z @ Wm3.T + bm3
    return (1.0 / (1.0 + np.exp(-logit))).astype(np.float32)


LAST = {"exec_ns": None, "profile": None, "trace": None}


def kernel(_trace=False, **inputs):
    import ml_dtypes
    global _PROG

    x_a = np.asarray(inputs["x_a"], np.float32)
    x_b = np.asarray(inputs["x_b"], np.float32)
    si = np.asarray(inputs["sample_indices"])
    W1 = np.asarray(inputs["W1"], np.float32); b1 = np.asarray(inputs["b1"], np.float32)
    W2 = np.asarray(inputs["W2"], np.float32); b2 = np.asarray(inputs["b2"], np.float32)
    W3 = np.asarray(inputs["W3"], np.float32); b3 = np.asarray(inputs["b3"], np.float32)
    Wm1 = np.asarray(inputs["Wm1"], np.float32); bm1 = np.asarray(inputs["bm1"], np.float32)
    g1 = np.asarray(inputs["g1"], np.float32); be1 = np.asarray(inputs["be1"], np.float32)
    Wm2 = np.asarray(inputs["Wm2"], np.float32); bm2 = np.asarray(inputs["bm2"], np.float32)
    g2 = np.asarray(inputs["g2"], np.float32); be2 = np.asarray(inputs["be2"], np.float32)
    Wm3 = np.asarray(inputs["Wm3"], np.float32); bm3 = np.asarray(inputs["bm3"], np.float32)

    counts = si[0].astype(np.int64)
    if not (x_a.shape == (1, CIN, L) and x_b.shape == (B, DVEC)
            and np.all(counts == P)):
        # segmentation differs from the uniform layout this kernel is
        # specialized for; fall back to a host reference implementation.
        return _numpy_reference(x_a, x_b, si, W1, b1, W2, b2, W3, b3,
                                Wm1, bm1, g1, be1, Wm2, bm2, g2, be2,
                                Wm3, bm3)

    from concourse.bass_utils import run_bass_kernel_spmd

    if _PROG is None:
        _PROG = _build_program()

    w1j = np.zeros((128, 128), np.float32)
    for j in range(4):
        w1j[32 * j:32 * j + 16, :] = W1.T
    w1j = w1j.astype(ml_dtypes.bfloat16)
    eye = np.eye(64, dtype=np.float32)
    segw = (np.vstack([eye, eye]) / P).astype(ml_dtypes.bfloat16)
    shared = {
        "w1j": w1j,
        "w2t": np.ascontiguousarray(W2.T).astype(ml_dtypes.bfloat16),
        "w3t": np.ascontiguousarray(W3.T).astype(ml_dtypes.bfloat16),
        "segw": segw,
        "wm1": np.ascontiguousarray(Wm1.T),
        "wm2a": np.ascontiguousarray(Wm2.T[0:128]),
        "wm2b": np.ascontiguousarray(Wm2.T[128:256]),
        "wm3": np.ascontiguousarray(Wm3.T),
        "b1": b1.reshape(128, 1),
        "b2": b2.reshape(128, 1),
        "b3p": np.concatenate([b3, b3]).reshape(128, 1),
        "bng": np.stack([g1[0:128], g1[128:256], g2], axis=1).astype(np.float32),
        "bnb": np.stack([be1[0:128], be1[128:256], be2], axis=1).astype(np.float32),
        "bm3": bm3.reshape(1, 1),
    }
    in_maps = []
    for c in range(N_CORES):
        m = dict(shared)
        m["x"] = _prep_core_x(x_a[0, :, LC * c:LC * (c + 1)])
        m["xbt"] = np.ascontiguousarray(x_b[E * c:E * (c + 1)].T)
        in_maps.append(m)

    res = run_bass_kernel_spmd(_PROG, in_maps, list(range(N_CORES)),
                               trace=_trace)
    if _trace:
        LAST["exec_ns"] = res.exec_time_ns
        LAST["profile"] = res.profile_json
        LAST["trace"] = (res.instructions_and_trace[1]
                         if res.instructions_and_trace else None)
    out = np.concatenate([res.results[c]["out"][0] for c in range(N_CORES)])
    return out.reshape(B, 1).astype(np.float32)


# revision 24
# speedup vs baseline: 1.0856x; 1.0427x over previous
"""Trainium2 Bass kernel for nn_DeepSetsEnsemble (segment_reduce).

Model: PhiNet (3x pointwise conv 16->128->128->64 with ReLU) over 524288
points, uniform segment-mean into 4096 events of 128 points, concat with
per-event features [4096, 32], then MLP 96->256->128->1 with training-mode
BatchNorm after the first two layers and a final sigmoid.

Distribution: data-parallel over events. Core c owns events
[512c, 512c+512) = points [65536c, 65536(c+1)). Params replicated.
BatchNorm needs full-batch statistics -> two tiny AllReduces (512/256
floats) across the 8 cores.

Device layout choices:
 - "p-major" point order per core (host-side relayout): tile t holds point
   index-within-event p=t for all 512 events. The L3 output tile is then
   [features, events], so the segment-sum becomes a PSUM-accumulated chain
   of contiguous matmuls (lhsT = stacked identities / 128), with no
   strided reads and no pipeline tail.
 - fp32r matmuls for layers 1/2 and the MLP (full PE rate, fp32 storage).
   Layer 3 and the segment-sum run in bf16: walrus rejects fp32r
   col-tiling, and M=64 needs col-tiling to pack two point-tiles into the
   128 PSUM partitions so drains use all lanes.
 - ReLU+bias drains PSUM->SBUF are the kernel bottleneck (1 elem/cycle/
   lane): split between ScalarE (activation) and VectorE (tensor_scalar
   add+max), biggest free dims PSUM banks allow.
 - bm1/bm2 are dropped: training-mode BN subtracts the batch mean, so any
   bias added before BN cancels exactly.
"""
import sys
sys.path.insert(0, "/opt/trn_rl_repo")
sys.path.insert(0, "/root/.axon_site/_ro/trn_rl_repo")

import numpy as np

# ---------------------------------------------------------------- constants
B = 4096
L = 524288
CIN = 16
DVEC = 32
PHI = [128, 128, 64]
MLP = [256, 128]
EPS = 1e-5

N_CORES = 8
E = B // N_CORES          # 512 events per core
P = L // B                # 128 points per event
LC = L // N_CORES         # 65536 points per core
N_TILES = LC // 512       # 128 point-tiles (one per p when p-major)
N_CHUNKS = 16             # 8 tiles per chunk
INV_B = 1.0 / B

_PROG = None  # compiled program cache (per process)


def _build_program():
    import concourse.bass as bass
    import concourse.tile as tile
    from concourse import bacc, mybir
    from concourse.alu_op_type import AluOpType
    from contextlib import ExitStack

    F32 = mybir.dt.float32
    F32R = mybir.dt.float32r
    BF16 = mybir.dt.bfloat16
    AF = mybir.ActivationFunctionType
    X = mybir.AxisListType.X

    nc = bacc.Bacc("TRN2", target_bir_lowering=False, debug=False,
                   num_devices=N_CORES)

    x_d = nc.dram_tensor("x", [64, 16384], BF16, kind="ExternalInput")
    xbt_d = nc.dram_tensor("xbt", [DVEC, E], F32R, kind="ExternalInput")
    w1j_d = nc.dram_tensor("w1j", [128, 128], BF16, kind="ExternalInput")
    w2t_d = nc.dram_tensor("w2t", [128, 128], BF16, kind="ExternalInput")
    w3t_d = nc.dram_tensor("w3t", [128, 64], BF16, kind="ExternalInput")
    segw_d = nc.dram_tensor("segw", [128, 64], BF16, kind="ExternalInput")
    wm1_d = nc.dram_tensor("wm1", [96, 256], F32R, kind="ExternalInput")
    wm2a_d = nc.dram_tensor("wm2a", [128, 128], F32R, kind="ExternalInput")
    wm2b_d = nc.dram_tensor("wm2b", [128, 128], F32R, kind="ExternalInput")
    wm3_d = nc.dram_tensor("wm3", [128, 1], F32R, kind="ExternalInput")
    b1_d = nc.dram_tensor("b1", [128, 1], F32, kind="ExternalInput")
    b2_d = nc.dram_tensor("b2", [128, 1], F32, kind="ExternalInput")
    b3p_d = nc.dram_tensor("b3p", [128, 1], F32, kind="ExternalInput")
    bng_d = nc.dram_tensor("bng", [128, 3], F32, kind="ExternalInput")
    bnb_d = nc.dram_tensor("bnb", [128, 3], F32, kind="ExternalInput")
    bm3_d = nc.dram_tensor("bm3", [1, 1], F32, kind="ExternalInput")
    out_d = nc.dram_tensor("out", [1, E], F32, kind="ExternalOutput")

    with tile.TileContext(nc) as tc, ExitStack() as ctx:
        wpool = ctx.enter_context(tc.tile_pool(name="wpool", bufs=1))
        xpool = ctx.enter_context(tc.tile_pool(name="xpool", bufs=4))
        hpool = ctx.enter_context(tc.tile_pool(name="hpool", bufs=4))
        h3pool = ctx.enter_context(tc.tile_pool(name="h3pool", bufs=6))
        mpool = ctx.enter_context(tc.tile_pool(name="mpool", bufs=1))
        pspool = ctx.enter_context(tc.tile_pool(name="pspool", bufs=1, space="PSUM"))
        dram = ctx.enter_context(tc.tile_pool(name="dram", bufs=1, space="DRAM"))

        # ---- weights / params to SBUF
        w1sb = wpool.tile([128, 128], BF16)
        w2sb = wpool.tile([128, 128], BF16)
        w3sb = wpool.tile([128, 64], BF16)
        segsb = wpool.tile([128, 64], BF16)
        wm1sb = wpool.tile([96, 256], F32R)
        wm2asb = wpool.tile([128, 128], F32R)
        wm2bsb = wpool.tile([128, 128], F32R)
        wm3sb = wpool.tile([128, 1], F32R)
        b1sb = wpool.tile([128, 1], F32)
        b2sb = wpool.tile([128, 1], F32)
        b3psb = wpool.tile([128, 1], F32)
        bngsb = wpool.tile([128, 3], F32)
        bnbsb = wpool.tile([128, 3], F32)
        bm3sb = wpool.tile([1, 1], F32)
        for sb, d in [(w1sb, w1j_d), (w2sb, w2t_d), (w3sb, w3t_d),
                      (segsb, segw_d), (wm1sb, wm1_d), (wm2asb, wm2a_d),
                      (wm2bsb, wm2b_d), (wm3sb, wm3_d), (b1sb, b1_d),
                      (b2sb, b2_d), (b3psb, b3p_d), (bngsb, bng_d),
                      (bnbsb, bnb_d), (bm3sb, bm3_d)]:
            nc.gpsimd.dma_start(sb[:], d[:])

        zt = mpool.tile([96, E], F32R)            # MLP input [means; xbt]
        nc.gpsimd.dma_start(zt[64:96, :], xbt_d[:])

        # force the sigmoid table set once, up front: it also contains
        # relu/square/copy, so no mid-kernel ACT_TABLE_LOAD ever fires.
        sgdum = wpool.tile([1, 1], F32)
        nc.scalar.activation(sgdum[:], bm3sb[:], AF.Sigmoid)

        # segment sums accumulate per-chunk in a rotating PSUM slot (dual
        # col-tiled chains), then a DVE add folds them into SBUF so all 8
        # banks stay available for the 4-slot drain rotation.
        segacc = mpool.tile([128, E], F32)
        segtmp = mpool.tile([64, E], F32)

        # warm the collective path early (overlaps phi compute)
        warm_in = dram.tile([1, 8], F32)
        warm_out = dram.tile([1, 8], F32)
        wtile = wpool.tile([1, 8], F32)
        nc.gpsimd.memset(wtile[:], 0.0)
        nc.sync.dma_start(warm_in[:], wtile[:])
        nc.gpsimd.collective_compute(
            "AllReduce", AluOpType.add,
            replica_groups=[list(range(N_CORES))],
            ins=[warm_in.opt()], outs=[warm_out.opt()])

        def drain(engine, dst, src, bias):
            # relu(src + bias): one PSUM->SBUF pass
            if engine == "act":
                nc.scalar.activation(dst, src, AF.Relu, bias=bias, scale=1.0)
            else:
                nc.vector.tensor_scalar(dst, src, bias, 0.0,
                                        AluOpType.add, AluOpType.max)

        # rotating PSUM drain groups: [128, 1024] (2 banks) x 4 bufs = all 8
        # banks; each group is drained by one engine, assigned greedily so
        # projected ACT/DVE loads stay balanced (DVE also absorbs the
        # per-chunk segment adds).
        load = {"act": 0.0, "dve": 0.0}

        def psum_group():
            g = pspool.tile([128, 1024], F32, name="grp", tag="grp", bufs=4)
            eng = "act" if load["act"] + 1147 <= load["dve"] + 1192 else "dve"
            load[eng] += 1147 if eng == "act" else 1192
            return g, eng

        for k in range(N_CHUNKS):
            xt = xpool.tile([128, 1024], BF16)
            for j in range(4):
                nc.sync.dma_start(xt[32 * j:32 * j + 16, :],
                                  x_d[16 * j:16 * j + 16,
                                      1024 * k:1024 * (k + 1)])

            h1sb = hpool.tile([128, 4096], BF16, name="h1sb", tag="h1sb")
            h2sb = hpool.tile([128, 4096], BF16, name="h2sb", tag="h2sb")
            h3sb = hpool.tile([128, 2048], BF16, name="h3sb", tag="h3sb")

            # L1: tiles t = 4u+j; 4 row-tiled MMs per u -> 2 groups
            for u in range(2):
                ga, ea = psum_group()
                gb, eb = psum_group()
                for j in range(4):
                    dst = (ga if j < 2 else gb)
                    nc.tensor.matmul(
                        dst[:, 512 * (j % 2):512 * (j % 2 + 1)],
                        w1sb[32 * j:32 * j + 16, :],
                        xt[32 * j:32 * j + 16, 512 * u:512 * (u + 1)],
                        start=True, stop=True, tile_position=(32 * j, 0))
                t0 = 4 * u
                drain(ea, h1sb[:, 512 * t0:512 * (t0 + 2)], ga[:], b1sb[:])
                drain(eb, h1sb[:, 512 * (t0 + 2):512 * (t0 + 4)], gb[:], b1sb[:])

            # L2: pairs of tiles per group
            for half in range(4):
                g, e = psum_group()
                for i, t in enumerate((2 * half, 2 * half + 1)):
                    nc.tensor.matmul(
                        g[:, 512 * i:512 * (i + 1)],
                        w2sb[:], h1sb[:, 512 * t:512 * (t + 1)],
                        start=True, stop=True)
                drain(e, h2sb[:, 1024 * half:1024 * (half + 1)], g[:], b2sb[:])

            # L3: two col-tiled pairs per group
            for q in range(2):
                g, e = psum_group()
                for i in range(2):
                    pr = 2 * q + i
                    t0, t1 = 2 * pr, 2 * pr + 1
                    nc.tensor.matmul(g[0:64, 512 * i:512 * (i + 1)], w3sb[:],
                                     h2sb[:, 512 * t0:512 * (t0 + 1)],
                                     start=True, stop=True,
                                     tile_position=(0, 0))
                    nc.tensor.matmul(g[64:128, 512 * i:512 * (i + 1)], w3sb[:],
                                     h2sb[:, 512 * t1:512 * (t1 + 1)],
                                     start=True, stop=True,
                                     tile_position=(0, 64))
                drain(e, h3sb[:, 1024 * q:1024 * (q + 1)], g[:], b3psb[:])

            sg, _ = psum_group()
            for pr in range(4):
                half = pr % 2
                nc.tensor.matmul(sg[64 * half:64 * (half + 1), 0:512],
                                 segsb[:], h3sb[:, 512 * pr:512 * (pr + 1)],
                                 start=(pr < 2), stop=(pr >= 2),
                                 tile_position=(0, 64 * half))
            if k == 0:
                nc.vector.tensor_copy(segacc[:], sg[:, 0:512])
            else:
                nc.vector.tensor_add(segacc[:], segacc[:], sg[:, 0:512])
            load["dve"] += 658

        # ---- MLP tail (events stay sharded; BN stats via AllReduce)
        nc.sync.dma_start(segtmp[:], segacc[64:128, :])
        nc.vector.tensor_add(zt[0:64, :], segacc[0:64, :], segtmp[:])

        y1p = pspool.tile([128, 1024], F32, name="y1p", tag="grp", bufs=4)
        stats = mpool.tile([128, 4], F32)
        sq_scr = mpool.tile([128, 1024], F32)
        for h in range(2):
            nc.tensor.matmul(y1p[:, 512 * h:512 * (h + 1)],
                             wm1sb[:, 128 * h:128 * (h + 1)], zt[:],
                             start=True, stop=True)
        # cross the engines over the two halves so they run in parallel
        nc.vector.tensor_reduce(stats[:, 0:1], y1p[:, 0:512], X, AluOpType.add)
        nc.scalar.activation(sq_scr[:, 512:1024], y1p[:, 512:1024],
                             AF.Square, accum_out=stats[:, 3:4])
        nc.vector.tensor_reduce(stats[:, 1:2], y1p[:, 512:1024], X, AluOpType.add)
        nc.scalar.activation(sq_scr[:, 0:512], y1p[:, 0:512],
                             AF.Square, accum_out=stats[:, 2:3])

        ar1_in = dram.tile([128, 4], F32)
        ar1_out = dram.tile([128, 4], F32)
        nc.sync.dma_start(ar1_in[:], stats[:])
        nc.gpsimd.collective_compute(
            "AllReduce", AluOpType.add,
            replica_groups=[list(range(N_CORES))],
            ins=[ar1_in.opt()], outs=[ar1_out.opt()])
        gst = mpool.tile([128, 4], F32)
        nc.sync.dma_start(gst[:], ar1_out[:])

        # scale/shift [128, 3]: cols 0,1 -> BN1 halves, col 2 -> BN2
        I32 = mybir.dt.int32
        mu = mpool.tile([128, 3], F32)
        var = mpool.tile([128, 3], F32)
        sd = mpool.tile([128, 3], F32)
        rs = mpool.tile([128, 3], F32)
        sc = mpool.tile([128, 3], F32)
        sh = mpool.tile([128, 3], F32)
        magic = mpool.tile([128, 3], I32)
        nc.gpsimd.memset(magic[:], 0x5F3759DF)

        def bn_params(c0, c1, sum_ap, sq_ap):
            # mu = sum/B ; var = sq/B - mu^2 ; sc = g * rsqrt(var+eps)
            # sh = be - mu*sc    (written into cols [c0:c1) of the tiles)
            # rsqrt: DVE-only (quake seed + 3 Newton steps) so no ACT sqrt
            # table set is ever needed.
            nc.vector.tensor_scalar_mul(mu[:, c0:c1], sum_ap, INV_B)
            nc.vector.tensor_scalar_mul(var[:, c0:c1], sq_ap, INV_B)
            nc.vector.tensor_mul(sd[:, c0:c1], mu[:, c0:c1], mu[:, c0:c1])
            nc.vector.tensor_sub(var[:, c0:c1], var[:, c0:c1], sd[:, c0:c1])
            u = var[:, c0:c1]
            nc.vector.tensor_scalar_add(u, u, EPS)
            r = rs[:, c0:c1]
            nc.vector.tensor_scalar(r.bitcast(I32), u.bitcast(I32), 1, None,
                                    AluOpType.arith_shift_right)
            nc.vector.tensor_sub(r.bitcast(I32), magic[:, c0:c1],
                                 r.bitcast(I32))
            t = sd[:, c0:c1]
            for _ in range(2):
                nc.vector.tensor_mul(t, r, r)
                nc.vector.tensor_mul(t, t, u)
                nc.vector.tensor_scalar(t, t, -0.5, 1.5,
                                        AluOpType.mult, AluOpType.add)
                nc.vector.tensor_mul(r, r, t)
            nc.vector.tensor_mul(sc[:, c0:c1], bngsb[:, c0:c1], r)
            nc.vector.tensor_mul(sh[:, c0:c1], mu[:, c0:c1], sc[:, c0:c1])
            nc.vector.tensor_sub(sh[:, c0:c1], bnbsb[:, c0:c1], sh[:, c0:c1])

        bn_params(0, 2, gst[:, 0:2], gst[:, 2:4])

        z1 = mpool.tile([128, 1024], F32R)
        for h in range(2):
            nc.scalar.activation(z1[:, 512 * h:512 * (h + 1)],
                                 y1p[:, 512 * h:512 * (h + 1)], AF.Relu,
                                 bias=sh[:, h:h + 1], scale=sc[:, h:h + 1])

        y2p = pspool.tile([128, 512], F32, name="y2p", tag="grp", bufs=4)
        nc.tensor.matmul(y2p[:], wm2asb[:], z1[:, 0:512], start=True, stop=False)
        nc.tensor.matmul(y2p[:], wm2bsb[:], z1[:, 512:1024], start=False, stop=True)
        st2 = mpool.tile([128, 2], F32)
        nc.vector.tensor_reduce(st2[:, 0:1], y2p[:], X, AluOpType.add)
        nc.scalar.activation(sq_scr[:, 0:512], y2p[:], AF.Square,
                             accum_out=st2[:, 1:2])

        ar2_in = dram.tile([128, 2], F32)
        ar2_out = dram.tile([128, 2], F32)
        nc.sync.dma_start(ar2_in[:], st2[:])
        nc.gpsimd.collective_compute(
            "AllReduce", AluOpType.add,
            replica_groups=[list(range(N_CORES))],
            ins=[ar2_in.opt()], outs=[ar2_out.opt()])
        gst2 = mpool.tile([128, 2], F32)
        nc.sync.dma_start(gst2[:], ar2_out[:])

        bn_params(2, 3, gst2[:, 0:1], gst2[:, 1:2])

        z2 = mpool.tile([128, 512], F32R)
        nc.scalar.activation(z2[:], y2p[:], AF.Relu,
                             bias=sh[:, 2:3], scale=sc[:, 2:3])

        lp = pspool.tile([1, 512], F32, name="lp", tag="grp", bufs=4)
        nc.tensor.matmul(lp[:], wm3sb[:], z2[:], start=True, stop=True)
        probs = mpool.tile([1, 512], F32)
        nc.scalar.activation(probs[:], lp[:], AF.Sigmoid,
                             bias=bm3sb[:], scale=1.0)
        nc.sync.dma_start(out_d[:], probs[:])

    nc.compile()
    return nc


# ------------------------------------------------------------- host helpers
def _prep_core_x(x_core):
    # x_core [16, 65536] fp32, natural order col = e*128 + p.
    # Output [64, 16384]: row 16j+c, col 1024k+512u+e  <->  (c, e, p=8k+4u+j)
    A = np.ascontiguousarray(x_core).reshape(CIN, E, P)
    A = A.reshape(CIN, E, 16, 2, 4)            # c, e, k, u, j
    Bm = A.transpose(4, 0, 2, 3, 1)            # j, c, k, u, e
    import ml_dtypes
    return np.ascontiguousarray(Bm.reshape(64, 16384)).astype(ml_dtypes.bfloat16)


def _numpy_reference(x_a, x_b, sample_indices, W1, b1, W2, b2, W3, b3,
                     Wm1, bm1, g1, be1, Wm2, bm2, g2, be2, Wm3, bm3):
    counts = sample_indices[0].astype(np.int64)
    bounds = np.concatenate([[0], np.cumsum(counts)])
    h = x_a[0]
    h = np.maximum(W1 @ h + b1[:, None], 0.0)
    h = np.maximum(W2 @ h + b2[:, None], 0.0)
    h = np.maximum(W3 @ h + b3[:, None], 0.0)
    Lh = h.shape[1]
    seg = np.searchsorted(bounds, np.arange(Lh), side="right") - 1
    sums = np.zeros((counts.shape[0], h.shape[0]), np.float32)
    valid = (seg >= 0) & (seg < counts.shape[0])
    np.add.at(sums, seg[valid], h.T[valid])
    means = sums / np.maximum(counts, 1)[:, None]
    z = np.concatenate([means, x_b.reshape(-1, DVEC)], axis=1)

    def bn(x, g, b):
        m = x.mean(0)
        v = x.var(0)
        return (x - m) / np.sqrt(v + EPS) * g + b

    z = np.maximum(bn(z @ Wm1.T + bm1, g1, be1), 0.0)
    z = np.maximum(bn(z @ Wm2.T + bm2, g2, be2), 0.0)
    logit = z @ Wm3.T + bm3
    return (1.0 / (1.0 + np.exp(-logit))).astype(np.float32)


LAST = {"exec_ns": None, "profile": None, "trace": None}


def kernel(_trace=False, **inputs):
    import ml_dtypes
    global _PROG

    x_a = np.asarray(inputs["x_a"], np.float32)
    x_b = np.asarray(inputs["x_b"], np.float32)
    si = np.asarray(inputs["sample_indices"])
    W1 = np.asarray(inputs["W1"], np.float32); b1 = np.asarray(inputs["b1"], np.float32)
    W2 = np.asarray(inputs["W2"], np.float32); b2 = np.asarray(inputs["b2"], np.float32)
    W3 = np.asarray(inputs["W3"], np.float32); b3 = np.asarray(inputs["b3"], np.float32)
    Wm1 = np.asarray(inputs["Wm1"], np.float32); bm1 = np.asarray(inputs["bm1"], np.float32)
    g1 = np.asarray(inputs["g1"], np.float32); be1 = np.asarray(inputs["be1"], np.float32)
    Wm2 = np.asarray(inputs["Wm2"], np.float32); bm2 = np.asarray(inputs["bm2"], np.float32)
    g2 = np.asarray(inputs["g2"], np.float32); be2 = np.asarray(inputs["be2"], np.float32)
    Wm3 = np.asarray(inputs["Wm3"], np.float32); bm3 = np.asarray(inputs["bm3"], np.float32)

    counts = si[0].astype(np.int64)
    if not (x_a.shape == (1, CIN, L) and x_b.shape == (B, DVEC)
            and np.all(counts == P)):
        # segmentation differs from the uniform layout this kernel is
        # specialized for; fall back to a host reference implementation.
        return _numpy_reference(x_a, x_b, si, W1, b1, W2, b2, W3, b3,
                                Wm1, bm1, g1, be1, Wm2, bm2, g2, be2,
                                Wm3, bm3)

    from concourse.bass_utils import run_bass_kernel_spmd

    if _PROG is None:
        _PROG = _build_program()

    w1j = np.zeros((128, 128), np.float32)
    for j in range(4):
        w1j[32 * j:32 * j + 16, :] = W1.T
    w1j = w1j.astype(ml_dtypes.bfloat16)
    eye = np.eye(64, dtype=np.float32)
    segw = (np.vstack([eye, eye]) / P).astype(ml_dtypes.bfloat16)
    shared = {
        "w1j": w1j,
        "w2t": np.ascontiguousarray(W2.T).astype(ml_dtypes.bfloat16),
        "w3t": np.ascontiguousarray(W3.T).astype(ml_dtypes.bfloat16),
        "segw": segw,
        "wm1": np.ascontiguousarray(Wm1.T),
        "wm2a": np.ascontiguousarray(Wm2.T[0:128]),
        "wm2b": np.ascontiguousarray(Wm2.T[128:256]),
        "wm3": np.ascontiguousarray(Wm3.T),
        "b1": b1.reshape(128, 1),
        "b2": b2.reshape(128, 1),
        "b3p": np.concatenate([b3, b3]).reshape(128, 1),
        "bng": np.stack([g1[0:128], g1[128:256], g2], axis=1).astype(np.float32),
        "bnb": np.stack([be1[0:128], be1[128:256], be2], axis=1).astype(np.float32),
        "bm3": bm3.reshape(1, 1),
    }
    in_maps = []
    for c in range(N_CORES):
        m = dict(shared)
        m["x"] = _prep_core_x(x_a[0, :, LC * c:LC * (c + 1)])
        m["xbt"] = np.ascontiguousarray(x_b[E * c:E * (c + 1)].T)
        in_maps.append(m)

    res = run_bass_kernel_spmd(_PROG, in_maps, list(range(N_CORES)),
                               trace=_trace)
    if _trace:
        LAST["exec_ns"] = res.exec_time_ns
        LAST["profile"] = res.profile_json
        LAST["trace"] = (res.instructions_and_trace[1]
                         if res.instructions_and_trace else None)
    out = np.concatenate([res.results[c]["out"][0] for c in range(N_CORES)])
    return out.reshape(B, 1).astype(np.float32)


# revision 25
# speedup vs baseline: 1.1478x; 1.0573x over previous
"""Trainium2 Bass kernel for nn_DeepSetsEnsemble (segment_reduce).

Model: PhiNet (3x pointwise conv 16->128->128->64 with ReLU) over 524288
points, uniform segment-mean into 4096 events of 128 points, concat with
per-event features [4096, 32], then MLP 96->256->128->1 with training-mode
BatchNorm after the first two layers and a final sigmoid.

Distribution: data-parallel over events. Core c owns events
[512c, 512c+512) = points [65536c, 65536(c+1)). Params replicated.
BatchNorm needs full-batch statistics -> two tiny AllReduces (512/256
floats) across the 8 cores.

Device layout choices:
 - "p-major" point order per core (host-side relayout): tile t holds point
   index-within-event p=t for all 512 events. The L3 output tile is then
   [features, events], so the segment-sum becomes a PSUM-accumulated chain
   of contiguous matmuls (lhsT = stacked identities / 128), with no
   strided reads and no pipeline tail.
 - fp32r matmuls for layers 1/2 and the MLP (full PE rate, fp32 storage).
   Layer 3 and the segment-sum run in bf16: walrus rejects fp32r
   col-tiling, and M=64 needs col-tiling to pack two point-tiles into the
   128 PSUM partitions so drains use all lanes.
 - ReLU+bias drains PSUM->SBUF are the kernel bottleneck (1 elem/cycle/
   lane): split between ScalarE (activation) and VectorE (tensor_scalar
   add+max), biggest free dims PSUM banks allow.
 - bm1/bm2 are dropped: training-mode BN subtracts the batch mean, so any
   bias added before BN cancels exactly.
"""
import sys
sys.path.insert(0, "/opt/trn_rl_repo")
sys.path.insert(0, "/root/.axon_site/_ro/trn_rl_repo")

import numpy as np

# ---------------------------------------------------------------- constants
B = 4096
L = 524288
CIN = 16
DVEC = 32
PHI = [128, 128, 64]
MLP = [256, 128]
EPS = 1e-5

N_CORES = 8
E = B // N_CORES          # 512 events per core
P = L // B                # 128 points per event
LC = L // N_CORES         # 65536 points per core
N_TILES = LC // 512       # 128 point-tiles (one per p when p-major)
N_CHUNKS = 16             # 8 tiles per chunk
INV_B = 1.0 / B

_PROG = None  # compiled program cache (per process)


def _build_program():
    import concourse.bass as bass
    import concourse.tile as tile
    from concourse import bacc, mybir
    from concourse.alu_op_type import AluOpType
    from contextlib import ExitStack

    F32 = mybir.dt.float32
    F32R = mybir.dt.float32r
    BF16 = mybir.dt.bfloat16
    AF = mybir.ActivationFunctionType
    X = mybir.AxisListType.X

    nc = bacc.Bacc("TRN2", target_bir_lowering=False, debug=False,
                   num_devices=N_CORES)

    x_d = nc.dram_tensor("x", [64, 16384], BF16, kind="ExternalInput")
    xbt_d = nc.dram_tensor("xbt", [DVEC, E], F32R, kind="ExternalInput")
    w1j_d = nc.dram_tensor("w1j", [128, 128], BF16, kind="ExternalInput")
    w2t_d = nc.dram_tensor("w2t", [128, 128], BF16, kind="ExternalInput")
    w3t_d = nc.dram_tensor("w3t", [128, 64], BF16, kind="ExternalInput")
    segw_d = nc.dram_tensor("segw", [128, 64], BF16, kind="ExternalInput")
    wm1_d = nc.dram_tensor("wm1", [96, 256], F32R, kind="ExternalInput")
    wm2a_d = nc.dram_tensor("wm2a", [128, 128], F32R, kind="ExternalInput")
    wm2b_d = nc.dram_tensor("wm2b", [128, 128], F32R, kind="ExternalInput")
    wm3_d = nc.dram_tensor("wm3", [128, 1], F32R, kind="ExternalInput")
    b1_d = nc.dram_tensor("b1", [128, 1], F32, kind="ExternalInput")
    b2_d = nc.dram_tensor("b2", [128, 1], F32, kind="ExternalInput")
    b3p_d = nc.dram_tensor("b3p", [128, 1], F32, kind="ExternalInput")
    bng_d = nc.dram_tensor("bng", [128, 3], F32, kind="ExternalInput")
    bnb_d = nc.dram_tensor("bnb", [128, 3], F32, kind="ExternalInput")
    bm3_d = nc.dram_tensor("bm3", [1, 1], F32, kind="ExternalInput")
    out_d = nc.dram_tensor("out", [1, E], F32, kind="ExternalOutput")

    with tile.TileContext(nc) as tc, ExitStack() as ctx:
        wpool = ctx.enter_context(tc.tile_pool(name="wpool", bufs=1))
        xpool = ctx.enter_context(tc.tile_pool(name="xpool", bufs=4))
        hpool = ctx.enter_context(tc.tile_pool(name="hpool", bufs=4))
        h3pool = ctx.enter_context(tc.tile_pool(name="h3pool", bufs=6))
        mpool = ctx.enter_context(tc.tile_pool(name="mpool", bufs=1))
        pspool = ctx.enter_context(tc.tile_pool(name="pspool", bufs=1, space="PSUM"))
        dram = ctx.enter_context(tc.tile_pool(name="dram", bufs=1, space="DRAM"))

        # ---- weights / params to SBUF
        w1sb = wpool.tile([128, 128], BF16)
        w2sb = wpool.tile([128, 128], BF16)
        w3sb = wpool.tile([128, 64], BF16)
        segsb = wpool.tile([128, 64], BF16)
        wm1sb = wpool.tile([96, 256], F32R)
        wm2asb = wpool.tile([128, 128], F32R)
        wm2bsb = wpool.tile([128, 128], F32R)
        wm3sb = wpool.tile([128, 1], F32R)
        b1sb = wpool.tile([128, 1], F32)
        b2sb = wpool.tile([128, 1], F32)
        b3psb = wpool.tile([128, 1], F32)
        bngsb = wpool.tile([128, 3], F32)
        bnbsb = wpool.tile([128, 3], F32)
        bm3sb = wpool.tile([1, 1], F32)
        for sb, d in [(w1sb, w1j_d), (w2sb, w2t_d), (w3sb, w3t_d),
                      (segsb, segw_d), (wm1sb, wm1_d), (wm2asb, wm2a_d),
                      (wm2bsb, wm2b_d), (wm3sb, wm3_d), (b1sb, b1_d),
                      (b2sb, b2_d), (b3psb, b3p_d), (bngsb, bng_d),
                      (bnbsb, bnb_d), (bm3sb, bm3_d)]:
            nc.gpsimd.dma_start(sb[:], d[:])

        zt = mpool.tile([96, E], F32R)            # MLP input [means; xbt]
        nc.gpsimd.dma_start(zt[64:96, :], xbt_d[:])

        # force the sigmoid table set once, up front: it also contains
        # relu/square/copy, so no mid-kernel ACT_TABLE_LOAD ever fires.
        sgdum = wpool.tile([1, 1], F32)
        nc.scalar.activation(sgdum[:], bm3sb[:], AF.Sigmoid)

        # segment sums accumulate per-chunk in a rotating PSUM slot (dual
        # col-tiled chains), then a DVE add folds them into SBUF so all 8
        # banks stay available for the 4-slot drain rotation.
        segacc = mpool.tile([128, E], F32)
        segtmp = mpool.tile([64, E], F32)

        # warm the collective path early (overlaps phi compute)
        warm_in = dram.tile([1, 8], F32)
        warm_out = dram.tile([1, 8], F32)
        wtile = wpool.tile([1, 8], F32)
        nc.gpsimd.memset(wtile[:], 0.0)
        nc.sync.dma_start(warm_in[:], wtile[:])
        nc.gpsimd.collective_compute(
            "AllReduce", AluOpType.add,
            replica_groups=[list(range(N_CORES))],
            ins=[warm_in.opt()], outs=[warm_out.opt()])

        def drain(engine, dst, src, bias):
            # relu(src + bias): one PSUM->SBUF pass
            if engine == "act":
                nc.scalar.activation(dst, src, AF.Relu, bias=bias, scale=1.0)
            else:
                nc.vector.tensor_scalar(dst, src, bias, 0.0,
                                        AluOpType.add, AluOpType.max)

        # rotating PSUM drain groups: [128, 1024] (2 banks) x 4 bufs = all 8
        # banks; each group is drained by one engine, assigned greedily so
        # projected ACT/DVE loads stay balanced (DVE also absorbs the
        # per-chunk segment adds).
        load = {"act": 0.0, "dve": 0.0}

        def psum_group():
            g = pspool.tile([128, 1024], F32, name="grp", tag="grp", bufs=4)
            eng = "act" if load["act"] + 1147 <= load["dve"] + 1192 else "dve"
            load[eng] += 1147 if eng == "act" else 1192
            return g, eng

        def emit_l1(k, st):
            xt = st["xt"]
            h1sb = st["h1sb"]
            for u in range(2):
                ga, ea = psum_group()
                gb, eb = psum_group()
                for j in range(4):
                    dst = (ga if j < 2 else gb)
                    nc.tensor.matmul(
                        dst[:, 512 * (j % 2):512 * (j % 2 + 1)],
                        w1sb[32 * j:32 * j + 16, :],
                        xt[32 * j:32 * j + 16, 512 * u:512 * (u + 1)],
                        start=True, stop=True, tile_position=(32 * j, 0))
                t0 = 4 * u
                drain(ea, h1sb[:, 512 * t0:512 * (t0 + 2)], ga[:], b1sb[:])
                drain(eb, h1sb[:, 512 * (t0 + 2):512 * (t0 + 4)], gb[:], b1sb[:])

        def emit_l2(k, st):
            h1sb, h2sb = st["h1sb"], st["h2sb"]
            for half in range(4):
                g, e = psum_group()
                for i, t in enumerate((2 * half, 2 * half + 1)):
                    nc.tensor.matmul(
                        g[:, 512 * i:512 * (i + 1)],
                        w2sb[:], h1sb[:, 512 * t:512 * (t + 1)],
                        start=True, stop=True)
                drain(e, h2sb[:, 1024 * half:1024 * (half + 1)], g[:], b2sb[:])

        def emit_l3(k, st):
            h2sb, h3sb = st["h2sb"], st["h3sb"]
            for q in range(2):
                g, e = psum_group()
                for i in range(2):
                    pr = 2 * q + i
                    t0, t1 = 2 * pr, 2 * pr + 1
                    nc.tensor.matmul(g[0:64, 512 * i:512 * (i + 1)], w3sb[:],
                                     h2sb[:, 512 * t0:512 * (t0 + 1)],
                                     start=True, stop=True,
                                     tile_position=(0, 0))
                    nc.tensor.matmul(g[64:128, 512 * i:512 * (i + 1)], w3sb[:],
                                     h2sb[:, 512 * t1:512 * (t1 + 1)],
                                     start=True, stop=True,
                                     tile_position=(0, 64))
                drain(e, h3sb[:, 1024 * q:1024 * (q + 1)], g[:], b3psb[:])
            sg, _ = psum_group()
            for pr in range(4):
                half = pr % 2
                nc.tensor.matmul(sg[64 * half:64 * (half + 1), 0:512],
                                 segsb[:], h3sb[:, 512 * pr:512 * (pr + 1)],
                                 start=(pr < 2), stop=(pr >= 2),
                                 tile_position=(0, 64 * half))
            if k == 0:
                nc.vector.tensor_copy(segacc[:], sg[:, 0:512])
            else:
                nc.vector.tensor_add(segacc[:], segacc[:], sg[:, 0:512])
            load["dve"] += 658

        states = {}
        for s in range(N_CHUNKS + 2):
            if s < N_CHUNKS:
                st = {}
                st["xt"] = xpool.tile([128, 1024], BF16, name="xt", tag="xt")
                st["h1sb"] = hpool.tile([128, 4096], BF16, name="h1sb", tag="h1sb")
                st["h2sb"] = hpool.tile([128, 4096], BF16, name="h2sb", tag="h2sb")
                st["h3sb"] = hpool.tile([128, 2048], BF16, name="h3sb", tag="h3sb")
                states[s] = st
                for j in range(4):
                    nc.sync.dma_start(st["xt"][32 * j:32 * j + 16, :],
                                      x_d[16 * j:16 * j + 16,
                                          1024 * s:1024 * (s + 1)])
                emit_l1(s, st)
            if 1 <= s <= N_CHUNKS:
                emit_l2(s - 1, states[s - 1])
            if 2 <= s:
                emit_l3(s - 2, states[s - 2])
                del states[s - 2]

        # ---- MLP tail (events stay sharded; BN stats via AllReduce)
        nc.sync.dma_start(segtmp[:], segacc[64:128, :])
        nc.vector.tensor_add(zt[0:64, :], segacc[0:64, :], segtmp[:])

        y1p = pspool.tile([128, 1024], F32, name="y1p", tag="grp", bufs=4)
        stats = mpool.tile([128, 4], F32)
        sq_scr = mpool.tile([128, 1024], F32)
        for h in range(2):
            nc.tensor.matmul(y1p[:, 512 * h:512 * (h + 1)],
                             wm1sb[:, 128 * h:128 * (h + 1)], zt[:],
                             start=True, stop=True)
        # cross the engines over the two halves so they run in parallel
        nc.vector.tensor_reduce(stats[:, 0:1], y1p[:, 0:512], X, AluOpType.add)
        nc.scalar.activation(sq_scr[:, 512:1024], y1p[:, 512:1024],
                             AF.Square, accum_out=stats[:, 3:4])
        nc.vector.tensor_reduce(stats[:, 1:2], y1p[:, 512:1024], X, AluOpType.add)
        nc.scalar.activation(sq_scr[:, 0:512], y1p[:, 0:512],
                             AF.Square, accum_out=stats[:, 2:3])

        ar1_in = dram.tile([128, 4], F32)
        ar1_out = dram.tile([128, 4], F32)
        nc.sync.dma_start(ar1_in[:], stats[:])
        nc.gpsimd.collective_compute(
            "AllReduce", AluOpType.add,
            replica_groups=[list(range(N_CORES))],
            ins=[ar1_in.opt()], outs=[ar1_out.opt()])
        gst = mpool.tile([128, 4], F32)
        nc.sync.dma_start(gst[:], ar1_out[:])

        # scale/shift [128, 3]: cols 0,1 -> BN1 halves, col 2 -> BN2
        I32 = mybir.dt.int32
        mu = mpool.tile([128, 3], F32)
        var = mpool.tile([128, 3], F32)
        sd = mpool.tile([128, 3], F32)
        rs = mpool.tile([128, 3], F32)
        sc = mpool.tile([128, 3], F32)
        sh = mpool.tile([128, 3], F32)
        magic = mpool.tile([128, 3], I32)
        nc.gpsimd.memset(magic[:], 0x5F3759DF)

        def bn_params(c0, c1, sum_ap, sq_ap):
            # mu = sum/B ; var = sq/B - mu^2 ; sc = g * rsqrt(var+eps)
            # sh = be - mu*sc    (written into cols [c0:c1) of the tiles)
            # rsqrt: DVE-only (quake seed + 3 Newton steps) so no ACT sqrt
            # table set is ever needed.
            nc.vector.tensor_scalar_mul(mu[:, c0:c1], sum_ap, INV_B)
            nc.vector.tensor_scalar_mul(var[:, c0:c1], sq_ap, INV_B)
            nc.vector.tensor_mul(sd[:, c0:c1], mu[:, c0:c1], mu[:, c0:c1])
            nc.vector.tensor_sub(var[:, c0:c1], var[:, c0:c1], sd[:, c0:c1])
            u = var[:, c0:c1]
            nc.vector.tensor_scalar_add(u, u, EPS)
            r = rs[:, c0:c1]
            nc.vector.tensor_scalar(r.bitcast(I32), u.bitcast(I32), 1, None,
                                    AluOpType.arith_shift_right)
            nc.vector.tensor_sub(r.bitcast(I32), magic[:, c0:c1],
                                 r.bitcast(I32))
            t = sd[:, c0:c1]
            for _ in range(2):
                nc.vector.tensor_mul(t, r, r)
                nc.vector.tensor_mul(t, t, u)
                nc.vector.tensor_scalar(t, t, -0.5, 1.5,
                                        AluOpType.mult, AluOpType.add)
                nc.vector.tensor_mul(r, r, t)
            nc.vector.tensor_mul(sc[:, c0:c1], bngsb[:, c0:c1], r)
            nc.vector.tensor_mul(sh[:, c0:c1], mu[:, c0:c1], sc[:, c0:c1])
            nc.vector.tensor_sub(sh[:, c0:c1], bnbsb[:, c0:c1], sh[:, c0:c1])

        bn_params(0, 2, gst[:, 0:2], gst[:, 2:4])

        z1 = mpool.tile([128, 1024], F32R)
        for h in range(2):
            nc.scalar.activation(z1[:, 512 * h:512 * (h + 1)],
                                 y1p[:, 512 * h:512 * (h + 1)], AF.Relu,
                                 bias=sh[:, h:h + 1], scale=sc[:, h:h + 1])

        y2p = pspool.tile([128, 512], F32, name="y2p", tag="grp", bufs=4)
        nc.tensor.matmul(y2p[:], wm2asb[:], z1[:, 0:512], start=True, stop=False)
        nc.tensor.matmul(y2p[:], wm2bsb[:], z1[:, 512:1024], start=False, stop=True)
        st2 = mpool.tile([128, 2], F32)
        nc.vector.tensor_reduce(st2[:, 0:1], y2p[:], X, AluOpType.add)
        nc.scalar.activation(sq_scr[:, 0:512], y2p[:], AF.Square,
                             accum_out=st2[:, 1:2])

        ar2_in = dram.tile([128, 2], F32)
        ar2_out = dram.tile([128, 2], F32)
        nc.sync.dma_start(ar2_in[:], st2[:])
        nc.gpsimd.collective_compute(
            "AllReduce", AluOpType.add,
            replica_groups=[list(range(N_CORES))],
            ins=[ar2_in.opt()], outs=[ar2_out.opt()])
        gst2 = mpool.tile([128, 2], F32)
        nc.sync.dma_start(gst2[:], ar2_out[:])

        bn_params(2, 3, gst2[:, 0:1], gst2[:, 1:2])

        z2 = mpool.tile([128, 512], F32R)
        nc.scalar.activation(z2[:], y2p[:], AF.Relu,
                             bias=sh[:, 2:3], scale=sc[:, 2:3])

        lp = pspool.tile([1, 512], F32, name="lp", tag="grp", bufs=4)
        nc.tensor.matmul(lp[:], wm3sb[:], z2[:], start=True, stop=True)
        probs = mpool.tile([1, 512], F32)
        nc.scalar.activation(probs[:], lp[:], AF.Sigmoid,
                             bias=bm3sb[:], scale=1.0)
        nc.sync.dma_start(out_d[:], probs[:])

    nc.compile()
    return nc


# ------------------------------------------------------------- host helpers
def _prep_core_x(x_core):
    # x_core [16, 65536] fp32, natural order col = e*128 + p.
    # Output [64, 16384]: row 16j+c, col 1024k+512u+e  <->  (c, e, p=8k+4u+j)
    A = np.ascontiguousarray(x_core).reshape(CIN, E, P)
    A = A.reshape(CIN, E, 16, 2, 4)            # c, e, k, u, j
    Bm = A.transpose(4, 0, 2, 3, 1)            # j, c, k, u, e
    import ml_dtypes
    return np.ascontiguousarray(Bm.reshape(64, 16384)).astype(ml_dtypes.bfloat16)


def _numpy_reference(x_a, x_b, sample_indices, W1, b1, W2, b2, W3, b3,
                     Wm1, bm1, g1, be1, Wm2, bm2, g2, be2, Wm3, bm3):
    counts = sample_indices[0].astype(np.int64)
    bounds = np.concatenate([[0], np.cumsum(counts)])
    h = x_a[0]
    h = np.maximum(W1 @ h + b1[:, None], 0.0)
    h = np.maximum(W2 @ h + b2[:, None], 0.0)
    h = np.maximum(W3 @ h + b3[:, None], 0.0)
    Lh = h.shape[1]
    seg = np.searchsorted(bounds, np.arange(Lh), side="right") - 1
    sums = np.zeros((counts.shape[0], h.shape[0]), np.float32)
    valid = (seg >= 0) & (seg < counts.shape[0])
    np.add.at(sums, seg[valid], h.T[valid])
    means = sums / np.maximum(counts, 1)[:, None]
    z = np.concatenate([means, x_b.reshape(-1, DVEC)], axis=1)

    def bn(x, g, b):
        m = x.mean(0)
        v = x.var(0)
        return (x - m) / np.sqrt(v + EPS) * g + b

    z = np.maximum(bn(z @ Wm1.T + bm1, g1, be1), 0.0)
    z = np.maximum(bn(z @ Wm2.T + bm2, g2, be2), 0.0)
    logit = z @ Wm3.T + bm3
    return (1.0 / (1.0 + np.exp(-logit))).astype(np.float32)


LAST = {"exec_ns": None, "profile": None, "trace": None}


def kernel(_trace=False, **inputs):
    import ml_dtypes
    global _PROG

    x_a = np.asarray(inputs["x_a"], np.float32)
    x_b = np.asarray(inputs["x_b"], np.float32)
    si = np.asarray(inputs["sample_indices"])
    W1 = np.asarray(inputs["W1"], np.float32); b1 = np.asarray(inputs["b1"], np.float32)
    W2 = np.asarray(inputs["W2"], np.float32); b2 = np.asarray(inputs["b2"], np.float32)
    W3 = np.asarray(inputs["W3"], np.float32); b3 = np.asarray(inputs["b3"], np.float32)
    Wm1 = np.asarray(inputs["Wm1"], np.float32); bm1 = np.asarray(inputs["bm1"], np.float32)
    g1 = np.asarray(inputs["g1"], np.float32); be1 = np.asarray(inputs["be1"], np.float32)
    Wm2 = np.asarray(inputs["Wm2"], np.float32); bm2 = np.asarray(inputs["bm2"], np.float32)
    g2 = np.asarray(inputs["g2"], np.float32); be2 = np.asarray(inputs["be2"], np.float32)
    Wm3 = np.asarray(inputs["Wm3"], np.float32); bm3 = np.asarray(inputs["bm3"], np.float32)

    counts = si[0].astype(np.int64)
    if not (x_a.shape == (1, CIN, L) and x_b.shape == (B, DVEC)
            and np.all(counts == P)):
        # segmentation differs from the uniform layout this kernel is
        # specialized for; fall back to a host reference implementation.
        return _numpy_reference(x_a, x_b, si, W1, b1, W2, b2, W3, b3,
                                Wm1, bm1, g1, be1, Wm2, bm2, g2, be2,
                                Wm3, bm3)

    from concourse.bass_utils import run_bass_kernel_spmd

    if _PROG is None:
        _PROG = _build_program()

    w1j = np.zeros((128, 128), np.float32)
    for j in range(4):
        w1j[32 * j:32 * j + 16, :] = W1.T
    w1j = w1j.astype(ml_dtypes.bfloat16)
    eye = np.eye(64, dtype=np.float32)
    segw = (np.vstack([eye, eye]) / P).astype(ml_dtypes.bfloat16)
    shared = {
        "w1j": w1j,
        "w2t": np.ascontiguousarray(W2.T).astype(ml_dtypes.bfloat16),
        "w3t": np.ascontiguousarray(W3.T).astype(ml_dtypes.bfloat16),
        "segw": segw,
        "wm1": np.ascontiguousarray(Wm1.T),
        "wm2a": np.ascontiguousarray(Wm2.T[0:128]),
        "wm2b": np.ascontiguousarray(Wm2.T[128:256]),
        "wm3": np.ascontiguousarray(Wm3.T),
        "b1": b1.reshape(128, 1),
        "b2": b2.reshape(128, 1),
        "b3p": np.concatenate([b3, b3]).reshape(128, 1),
        "bng": np.stack([g1[0:128], g1[128:256], g2], axis=1).astype(np.float32),
        "bnb": np.stack([be1[0:128], be1[128:256], be2], axis=1).astype(np.float32),
        "bm3": bm3.reshape(1, 1),
    }
    in_maps = []
    for c in range(N_CORES):
        m = dict(shared)
        m["x"] = _prep_core_x(x_a[0, :, LC * c:LC * (c + 1)])
        m["xbt"] = np.ascontiguousarray(x_b[E * c:E * (c + 1)].T)
        in_maps.append(m)

    res = run_bass_kernel_spmd(_PROG, in_maps, list(range(N_CORES)),
                               trace=_trace)
    if _trace:
        LAST["exec_ns"] = res.exec_time_ns
        LAST["profile"] = res.profile_json
        LAST["trace"] = (res.instructions_and_trace[1]
                         if res.instructions_and_trace else None)
    out = np.concatenate([res.results[c]["out"][0] for c in range(N_CORES)])
    return out.reshape(B, 1).astype(np.float32)
